# revision 1
# baseline (speedup 1.0000x reference)
"""Trainium2 Bass kernel for nn_CustomBSplineLayer.

Math: out[b,o] = sum_{i,g} coeff[o,i,g] * w[o,i] * s_g(clip(x[b,i], -1, 1))
where s_g is a cubic B-spline basis on uniform knots (spacing h = 2/7,
centers linspace(-1, 15/7, 12), 8 basis functions, order 3; s_7 == 0 on the
clipped domain).

Uniform-knot truncated-power identity with t = (clip(x,-1,1)+1)/h in [0,7]:
    s_g = (1/6) * sum_{r=0..4} w5[r] * V_{g+r},   V_q = relu(t-q)^3,  q=0..6
so out = sum_{q,i} P_q[b,i] * H[(q,i), o] for ANY plane basis P that spans the
{V_q} (coefficients H solved exactly on host).  The PE runs float32r (full
rate; true fp32 is 4x slower) which rounds each product at ~2^-12 relative, so
per-plane error scales with |P_q|*|H_q|.  Raw V planes (|V|<=343) give ~1.2e-2
relative error; folding differences of neighbouring cubes bounds the planes:
    D1_q = V_q - V_{q+1} (<=127),   W2_q = D1_q - D1_{q+1} (<=36)
A mixed basis tuned per-plane to the measured error profile gives ~1.2e-3 at
only 8 extra subtracts per i-block:
    P = {W2_0, W2_1, W2_2, D1_3, D1_4, V_5, V_6}

Layout (data-parallel over batch, 8 cores x 1024 rows):
  - x pre-transposed on host: xt [512 i, 1024 b] per core, i on partitions.
  - planes per (i-block, q): [128, 1024] tiles; matmul lhsT slices are
    [128 K, 128 M=batch] column windows; rhs H tiles [128, 512 o] (f32r).
  - K order kt = ib*7 + q matches plane production order.
  - PSUM [128 b, 512 o] x 8 banks accumulate all 28 k-tiles.
  - Engine split: ScalarE does relu+square (+psum drains), VectorE does cubes
    and f32r-facing folds/casts, GpSimd does interior fp32 D1 folds.
  - DVE ops READING float32r tiles round to ~12 bits (measured), so every
    fold input stays fp32; f32r appears only on op OUTPUTS (bit-benign) or
    via cheap copy-casts.
"""

import numpy as np

import concourse.mybir as mybir
from concourse import bacc
import concourse.tile as tile
from concourse.bass_utils import run_bass_kernel_spmd

F32 = mybir.dt.float32
F32R = mybir.dt.float32r
AOT = mybir.AluOpType
ACTF = mybir.ActivationFunctionType

N_CORES = 8
BATCH, I, O, G = 8192, 512, 512, 8
BC = BATCH // N_CORES          # 1024 batch rows per core
Q = 7                          # planes q = 0..6
IB = I // 128                  # 4 i-blocks
KT = Q * IB                    # 28 k-tiles of 128
NBB = BC // 128                # 8 batch blocks of 128

# plane level per q: 0 = raw V, 1 = D1, 2 = W2
LEVELS = (2, 2, 2, 1, 1, 0, 0)
SQ_DVE = 6         # how many of the 28 squares run on DVE (rest ScalarE)

_programs = {}


def _build_program(knobs=(LEVELS, SQ_DVE)):
    levels, sq_dve = knobs
    nc = bacc.Bacc("TRN2", target_bir_lowering=False, debug=False,
                   num_devices=N_CORES)
    xt_d = nc.dram_tensor("xt", [I, BC], F32, kind="ExternalInput").ap()
    h2_d = nc.dram_tensor("h2", [KT * 128, O], F32R, kind="ExternalInput").ap()
    qb_d = nc.dram_tensor("qb", [128, 8], F32, kind="ExternalInput").ap()
    out_d = nc.dram_tensor("out", [BC, O], F32, kind="ExternalOutput").ap()

    assert levels == (2, 2, 2, 1, 1, 0, 0), "kernel body is specialized"

    with tile.TileContext(nc) as tc:
        with tc.tile_pool(name="g", bufs=1) as gpool, \
             tc.tile_pool(name="x", bufs=2) as xpool, \
             tc.tile_pool(name="v", bufs=1) as vpool, \
             tc.tile_pool(name="tmp", bufs=3) as tpool, \
             tc.tile_pool(name="o", bufs=4) as opool, \
             tc.tile_pool(name="ps", bufs=1, space="PSUM") as pspool:

            qb_s = gpool.tile([128, 8], F32)
            nc.sync.dma_start(out=qb_s[:], in_=qb_d[:])

            h2_s = gpool.tile([128, KT, O], F32R)
            for ib in range(IB):
                nc.sync.dma_start(
                    out=h2_s[:, ib * Q:(ib + 1) * Q, :],
                    in_=h2_d[ib * Q * 128:(ib + 1) * Q * 128, :].rearrange(
                        "(kt p) o -> p kt o", p=128))

            psums = [pspool.tile([128, O], F32, name=f"ps{bb}", tag=f"ps{bb}")
                     for bb in range(NBB)]

            state = {"sq_dve": sq_dve}

            def mk_plane(ib, q, lhs):
                kt = ib * Q + q
                rhs = h2_s[:, kt, :]
                for bb in range(NBB):
                    nc.tensor.matmul(psums[bb][:],
                                     lhs[:, bb * 128:(bb + 1) * 128],
                                     rhs,
                                     start=(kt == 0), stop=(kt == KT - 1))

            for ib in range(IB):
                xs = xpool.tile([128, BC], F32, tag="x")
                nc.sync.dma_start(out=xs[:], in_=xt_d[ib * 128:(ib + 1) * 128, :])
                tp = xpool.tile([128, BC], F32, tag="tp")
                nc.vector.tensor_scalar(out=tp[:], in0=xs[:], scalar1=3.5,
                                        scalar2=3.5, op0=AOT.mult, op1=AOT.min)

                v = {}
                d1 = {}

                def cube(q, dtype=F32, tag="v", bufs=4):
                    qq = float(q) - 3.5
                    a = tpool.tile([128, BC], F32, tag="a")
                    nc.scalar.activation(a[:], tp[:], ACTF.Relu,
                                         bias=qb_s[:, q:q + 1], scale=1.0)
                    sq = tpool.tile([128, BC], F32, tag="sq")
                    if state["sq_dve"] > 0 and q == 3:
                        state["sq_dve"] -= 1
                        nc.vector.scalar_tensor_tensor(
                            out=sq[:], in0=tp[:], scalar=qq, in1=a[:],
                            op0=AOT.subtract, op1=AOT.mult)
                    else:
                        nc.scalar.activation(sq[:], a[:], ACTF.Square)
                    vq = vpool.tile([128, BC], dtype, tag=tag, bufs=bufs,
                                    name=f"{tag}_{ib}_{q}")
                    nc.vector.scalar_tensor_tensor(
                        out=vq[:], in0=tp[:], scalar=qq, in1=sq[:],
                        op0=AOT.subtract, op1=AOT.mult)
                    return vq

                def gp_sub(name, x0, x1):
                    dq = vpool.tile([128, BC], F32, tag="d1", bufs=3,
                                    name=f"{name}_{ib}")
                    nc.gpsimd.tensor_tensor(out=dq[:], in0=x0[:], in1=x1[:],
                                            op=AOT.subtract)
                    return dq

                def dve_sub_r(name, x0, x1):
                    wq = vpool.tile([128, BC], F32R, tag="w2", bufs=4,
                                    name=f"{name}_{ib}")
                    nc.vector.tensor_tensor(out=wq[:], in0=x0[:], in1=x1[:],
                                            op=AOT.subtract)
                    return wq

                for q in range(2):
                    v[q] = cube(q)
                d1[0] = gp_sub("d1_0", v[0], v[1])
                v[2] = cube(2)
                d1[1] = gp_sub("d1_1", v[1], v[2])
                mk_plane(ib, 0, dve_sub_r("w2_0", d1[0], d1[1]))
                v[3] = cube(3)
                d1[2] = gp_sub("d1_2", v[2], v[3])
                mk_plane(ib, 1, dve_sub_r("w2_1", d1[1], d1[2]))
                v[4] = cube(4)
                d1[3] = gp_sub("d1_3", v[3], v[4])
                mk_plane(ib, 2, dve_sub_r("w2_2", d1[2], d1[3]))
                # plane 3 = D1_3 (f32r copy-cast of the fp32 fold output)
                p3 = vpool.tile([128, BC], F32R, tag="w2", bufs=4,
                                name=f"p3_{ib}")
                nc.vector.tensor_copy(out=p3[:], in_=d1[3][:])
                mk_plane(ib, 3, p3)
                v[5] = cube(5)
                # plane 4 = D1_4 = v4 - v5 (f32r out directly; plane-only)
                mk_plane(ib, 4, dve_sub_r("d1_4", v[4], v[5]))
                # plane 5 = V_5 (f32r copy-cast)
                p5 = vpool.tile([128, BC], F32R, tag="w2", bufs=4,
                                name=f"p5_{ib}")
                nc.vector.tensor_copy(out=p5[:], in_=v[5][:])
                mk_plane(ib, 5, p5)
                # plane 6 = V_6, cube written straight to f32r
                mk_plane(ib, 6, cube(6, dtype=F32R, tag="w2", bufs=4))

            for bb in range(NBB):
                o = opool.tile([128, O], F32, tag="o")
                nc.scalar.copy(o[:], psums[bb][:])
                nc.sync.dma_start(out=out_d[bb * 128:(bb + 1) * 128, :], in_=o[:])

    nc.compile()
    return nc


def _get_program(knobs=(LEVELS, SQ_DVE)):
    if knobs not in _programs:
        _programs[knobs] = _build_program(knobs)
    return _programs[knobs]


_STENS = {0: (1.0,), 1: (1.0, -1.0), 2: (1.0, -2.0, 1.0)}


def _host_prep(x, weights, coefficients, levels=LEVELS):
    x = np.ascontiguousarray(np.asarray(x, dtype=np.float32))
    weights = np.asarray(weights, dtype=np.float32)
    coefficients = np.asarray(coefficients, dtype=np.float32)

    # raw truncated-power coefficients G_q = sum_g w5[q-g]/6 * C2_g  (g<=6)
    c2 = coefficients.astype(np.float64) * weights.astype(np.float64)[:, :, None]
    c2 = c2.transpose(2, 1, 0)[:Q]                 # [7, I, O]
    w5 = np.array([1.0, -4.0, 6.0, -4.0, 1.0])
    graw = np.zeros((Q, I, O), dtype=np.float64)
    for q in range(Q):
        for g in range(Q):
            r = q - g
            if 0 <= r <= 4:
                graw[q] += (w5[r] / 6.0) * c2[g]
    # planes P = A V  =>  coefficients H = A^{-T} G (exact basis change)
    A = np.zeros((Q, Q))
    for q in range(Q):
        for u, s in enumerate(_STENS[levels[q]]):
            if q + u < Q:
                A[q, q + u] = s
    h = np.einsum('pq,qio->pio', np.linalg.inv(A).T, graw)
    # device row order kt = ib*7 + q
    h2k = np.empty((KT, 128, O), dtype=np.float32)
    for ib in range(IB):
        for q in range(Q):
            h2k[ib * Q + q] = h[q, ib * 128:(ib + 1) * 128, :]
    h2k = np.ascontiguousarray(h2k.reshape(KT * 128, O))

    xt = np.ascontiguousarray(x.T)                 # [I, B]
    qb = np.tile((3.5 - np.arange(8, dtype=np.float32))[None, :], (128, 1))

    in_maps = []
    for c in range(N_CORES):
        in_maps.append({
            "xt": np.ascontiguousarray(xt[:, c * BC:(c + 1) * BC]),
            "h2": h2k,
            "qb": qb,
        })
    return in_maps


def _run(x, weights, coefficients, knobs=(LEVELS, SQ_DVE), **spmd_kwargs):
    nc = _get_program(knobs)
    in_maps = _host_prep(x, weights, coefficients, knobs[0])
    res = run_bass_kernel_spmd(nc, in_maps, list(range(N_CORES)), **spmd_kwargs)
    out = np.concatenate([res.results[c]["out"] for c in range(N_CORES)], axis=0)
    return out.astype(np.float32), res


def kernel(x, weights, coefficients):
    out, _ = _run(x, weights, coefficients)
    return out



# revision 2
# speedup vs baseline: 1.8682x; 1.8682x over previous
"""Trainium2 Bass kernel for nn_CustomBSplineLayer.

Math: out[b,o] = sum_{i,g} coeff[o,i,g] * w[o,i] * s_g(clip(x[b,i], -1, 1))
where s_g is a cubic B-spline basis on uniform knots (t = 3.5*(x+1) in [0,7],
8 basis functions; s_7 == 0 on the clipped domain).

Uniform-knot truncated-power identity: with V_q = relu(t-q)^3, the layer is
out = sum_{q,i} P_q[b,i] * H[(q,i), o] for ANY plane basis P spanning {V_q}
(H solved exactly on host).  The PE runs float32r (full rate) which rounds
each product at ~2^-12 relative, so per-plane error scales with |P_q|*|H_q|.
First-difference planes D1_q = V_q - V_{q+1} (<=127; V_7 := 0) measure ~5e-3
relative output error -- well under the 2e-2 gate -- and have a key property:

    D1_q is a function of a_q = relu(t-q) ALONE:
        D1_q = m^3 + 3*(a_q - m)*a_q,   m = min(a_q, 1)
    (for t>=q+1 this is 3s^2-3s+1 with s=t-q; for t in [q,q+1] it's s^3).

So each plane needs exactly TWO on-chip ops: one ScalarE relu (free bias
shift) and one fused custom DVE instruction (D1CUBE_ANT, registered below,
5 ALU stages).  No folds, no gpsimd, no squares: per i-block the old kernel's
~30 elementwise ops become 14.  The clamp tp = min(3.5x, 3.5) is precomputed
on host (t<0 needs no clamp: every plane vanishes there via the relu).

Layout (data-parallel over batch, 8 cores x 1024 rows):
  - xt = host tp, pre-transposed: [512 i, 1024 b] per core, i on partitions.
  - planes per (i-block, q): [128, 1024] f32r tiles; matmul lhsT slices are
    [128 K, 128 M=batch] column windows; rhs H tiles [128, 512 o] (f32r).
  - h2 DMA'd in 28 per-kt chunks so the first matmul can start ~1us in.
  - PSUM [128 b, 512 o] x 8 banks accumulate all 28 k-tiles.
"""

import numpy as np

import concourse.mybir as mybir
from concourse import bacc
import concourse.tile as tile
from concourse.bass_utils import run_bass_kernel_spmd
from concourse import dve_ops as _dops
from concourse.dve_spec import Spec, Src0, C0, One, minn, sq
from concourse.dve_spec import lower as _dve_lower
from concourse.dve_uop import DveOpSpec as _DveOpSpec

F32 = mybir.dt.float32
F32R = mybir.dt.float32r
ACTF = mybir.ActivationFunctionType

N_CORES = 8
BATCH, I, O, G = 8192, 512, 512, 8
BC = BATCH // N_CORES          # 1024 batch rows per core
Q = 7                          # planes q = 0..6
IB = I // 128                  # 4 i-blocks
KT = Q * IB                    # 28 k-tiles of 128
NBB = BC // 128                # 8 batch blocks of 128


def _register_d1cube():
    """Register the fused plane op: out = m^3 + s0*(a-m)*a, m = min(a, 1).

    With a = relu(t-q) >= 0 and s0 = 3.0 this is exactly
    D1_q(t) = relu(t-q)^3 - relu(t-q-1)^3 for t <= q+... (all t; V_{q+1}
    is a function of a_q since relu(t-q-1) = relu(a_q - 1))."""
    name = "D1CUBE_ANT"
    for op in _dops.OPS:
        if op.name == name:
            return op

    def _ref(in0, in1, s0, s1, imm2):
        a = in0.astype(np.float32)
        m = np.minimum(a, np.float32(1.0))
        return (m * m * m + (a - m) * a * np.float32(s0)).astype(np.float32)

    m = minn(Src0, One)
    spec = Spec(body=sq(m) * m + (Src0 - m) * Src0 * C0, reference=_ref)
    opcode = _dops._CUSTOM_DVE_ROW_BASE + len(_dops.OPS)
    assert opcode < 0x20
    shas = {}
    for ver in ("v3", "v4"):
        try:
            shas[ver] = _DveOpSpec(
                name=name, opcode=opcode, uops=_dve_lower(spec, ver=ver),
                rd1_en=False).sha(ver)
        except Exception:
            pass
    op = _dops.DveOp(name, spec, subdim=False, uops_sha=shas)
    _dops.OPS.append(op)
    _dops.CUSTOM_DVE_SPECS[name] = spec
    _dops._SUB_OPCODE_FOR_NAME[name] = opcode
    return op


D1CUBE = _register_d1cube()

_programs = {}


def _build_program():
    nc = bacc.Bacc("TRN2", target_bir_lowering=False, debug=False,
                   num_devices=N_CORES)
    xt_d = nc.dram_tensor("xt", [I, BC], F32, kind="ExternalInput").ap()
    h2_d = nc.dram_tensor("h2", [KT * 128, O], F32R, kind="ExternalInput").ap()
    qb_d = nc.dram_tensor("qb", [128, 8], F32, kind="ExternalInput").ap()
    out_d = nc.dram_tensor("out", [BC, O], F32, kind="ExternalOutput").ap()

    with tile.TileContext(nc) as tc:
        with tc.tile_pool(name="g", bufs=1) as gpool, \
             tc.tile_pool(name="x", bufs=4) as xpool, \
             tc.tile_pool(name="a", bufs=4) as apool, \
             tc.tile_pool(name="p", bufs=6) as ppool, \
             tc.tile_pool(name="o", bufs=4) as opool, \
             tc.tile_pool(name="ps", bufs=1, space="PSUM") as pspool:

            qb_s = gpool.tile([128, 8], F32)
            nc.sync.dma_start(out=qb_s[:], in_=qb_d[:])

            xs = []
            for ib in range(IB):
                x_t = xpool.tile([128, BC], F32, name=f"xs{ib}", tag=f"xs{ib}")
                nc.sync.dma_start(out=x_t[:],
                                  in_=xt_d[ib * 128:(ib + 1) * 128, :])
                xs.append(x_t)

            # h2 in per-kt chunks, in consumption order
            h2_s = gpool.tile([128, KT, O], F32R)
            for kt in range(KT):
                nc.sync.dma_start(out=h2_s[:, kt, :],
                                  in_=h2_d[kt * 128:(kt + 1) * 128, :])

            psums = [pspool.tile([128, O], F32, name=f"ps{bb}", tag=f"ps{bb}")
                     for bb in range(NBB)]

            for ib in range(IB):
                for q in range(Q):
                    kt = ib * Q + q
                    a = apool.tile([128, BC], F32, tag="a")
                    nc.scalar.activation(a[:], xs[ib][:], ACTF.Relu,
                                         bias=qb_s[:, q:q + 1], scale=1.0)
                    p = ppool.tile([128, BC], F32R, tag="p")
                    nc.vector._custom_dve(D1CUBE, out=p[:], in0=a[:], s0=3.0)
                    rhs = h2_s[:, kt, :]
                    for bb in range(NBB):
                        nc.tensor.matmul(psums[bb][:],
                                         p[:, bb * 128:(bb + 1) * 128],
                                         rhs,
                                         start=(kt == 0), stop=(kt == KT - 1))

            for bb in range(NBB):
                o = opool.tile([128, O], F32, tag="o")
                nc.scalar.copy(o[:], psums[bb][:])
                nc.sync.dma_start(out=out_d[bb * 128:(bb + 1) * 128, :],
                                  in_=o[:])

    nc.compile()
    return nc


def _get_program():
    if "p" not in _programs:
        _programs["p"] = _build_program()
    return _programs["p"]


def _host_prep(x, weights, coefficients):
    x = np.asarray(x, dtype=np.float32)
    weights = np.asarray(weights, dtype=np.float32)
    coefficients = np.asarray(coefficients, dtype=np.float32)

    # raw truncated-power coefficients G_q = sum_g w5[q-g]/6 * C2_g
    c2 = coefficients.astype(np.float64) * weights.astype(np.float64)[:, :, None]
    c2 = c2.transpose(2, 1, 0)                     # [G, I, O]
    w5 = np.array([1.0, -4.0, 6.0, -4.0, 1.0]) / 6.0
    graw = np.zeros((Q, I, O), dtype=np.float64)
    for q in range(Q):
        for g in range(G):
            r = q - g
            if 0 <= r <= 4:
                graw[q] += w5[r] * c2[g]
    # planes P_q = D1_q = V_q - V_{q+1} (V_7 := 0)  =>  H = A^{-T} G
    A = np.eye(Q)
    A[np.arange(Q - 1), np.arange(1, Q)] = -1.0
    h = np.einsum('pq,qio->pio', np.linalg.inv(A).T, graw)
    # device row order kt = ib*7 + q
    h2k = np.empty((KT, 128, O), dtype=np.float32)
    for ib in range(IB):
        for q in range(Q):
            h2k[ib * Q + q] = h[q, ib * 128:(ib + 1) * 128, :]
    h2k = np.ascontiguousarray(h2k.reshape(KT * 128, O))

    # tp = min(3.5*x, 3.5): t<0 needs no clamp (relu zeroes every plane)
    tp = np.minimum(3.5 * x, np.float32(3.5)).astype(np.float32)
    xt = np.ascontiguousarray(tp.T)                # [I, B]
    qb = np.tile((3.5 - np.arange(8, dtype=np.float32))[None, :], (128, 1))

    in_maps = []
    for c in range(N_CORES):
        in_maps.append({
            "xt": np.ascontiguousarray(xt[:, c * BC:(c + 1) * BC]),
            "h2": h2k,
            "qb": qb,
        })
    return in_maps


def _run(x, weights, coefficients, **spmd_kwargs):
    nc = _get_program()
    in_maps = _host_prep(x, weights, coefficients)
    res = run_bass_kernel_spmd(nc, in_maps, list(range(N_CORES)), **spmd_kwargs)
    out = np.concatenate([res.results[c]["out"] for c in range(N_CORES)], axis=0)
    return out.astype(np.float32), res


def kernel(x, weights, coefficients):
    out, _ = _run(x, weights, coefficients)
    return out


# revision 3
# speedup vs baseline: 2.0111x; 1.0765x over previous
"""Trainium2 Bass kernel for nn_CustomBSplineLayer.

Math: out[b,o] = sum_{i,g} coeff[o,i,g] * w[o,i] * s_g(clip(x[b,i], -1, 1))
where s_g is a cubic B-spline basis on uniform knots (t = 3.5*(x+1) in [0,7],
8 basis functions; s_7 == 0 on the clipped domain).

Uniform-knot truncated-power identity: with V_q = relu(t-q)^3, the layer is
out = sum_{q,i} P_q[b,i] * H[(q,i), o] for ANY plane basis P spanning {V_q}
(H solved exactly on host).  The PE runs float32r (full rate) which rounds
each product at ~2^-12 relative, so per-plane error scales with |P_q|*|H_q|.
First-difference planes D1_q = V_q - V_{q+1} (<=127; V_7 := 0) measure ~5e-3
relative output error -- well under the 2e-2 gate -- and have a key property:

    D1_q is a function of a_q = relu(t-q) ALONE:
        D1_q = m^3 + 3*(a_q - m)*a_q,   m = min(a_q, 1)
    (for t>=q+1 this is 3s^2-3s+1 with s=t-q; for t in [q,q+1] it's s^3).

So each plane needs exactly TWO on-chip ops: one ScalarE relu (free bias
shift) and one fused custom DVE instruction (D1CUBE_ANT, registered below,
5 ALU stages).  No folds, no gpsimd, no squares: per i-block the old kernel's
~30 elementwise ops become 14.  The clamp tp = min(3.5x, 3.5) is precomputed
on host (t<0 needs no clamp: every plane vanishes there via the relu).

Layout (data-parallel over batch, 8 cores x 1024 rows):
  - xt = host tp, pre-transposed: [512 i, 1024 b] per core, i on partitions.
  - planes per (i-block, q): [128, 1024] f32r tiles; matmul lhsT slices are
    [128 K, 128 M=batch] column windows; rhs H tiles [128, 512 o] (f32r).
  - h2 DMA'd in 28 per-kt chunks so the first matmul can start ~1us in.
  - PSUM [128 b, 512 o] x 8 banks accumulate all 28 k-tiles.
"""

import numpy as np

import concourse.mybir as mybir
from concourse import bacc
import concourse.tile as tile
from concourse.bass_utils import run_bass_kernel_spmd
from concourse import dve_ops as _dops
from concourse.dve_spec import Spec, Src0, C0, One, minn, sq
from concourse.dve_spec import lower as _dve_lower
from concourse.dve_uop import DveOpSpec as _DveOpSpec

F32 = mybir.dt.float32
F32R = mybir.dt.float32r
ACTF = mybir.ActivationFunctionType

N_CORES = 8
BATCH, I, O, G = 8192, 512, 512, 8
BC = BATCH // N_CORES          # 1024 batch rows per core
Q = 7                          # planes q = 0..6
IB = I // 128                  # 4 i-blocks
KT = Q * IB                    # 28 k-tiles of 128
NBB = BC // 128                # 8 batch blocks of 128


def _register_d1cube():
    """Register the fused plane op: out = m^3 + s0*(a-m)*a, m = min(a, 1).

    With a = relu(t-q) >= 0 and s0 = 3.0 this is exactly
    D1_q(t) = relu(t-q)^3 - relu(t-q-1)^3 for t <= q+... (all t; V_{q+1}
    is a function of a_q since relu(t-q-1) = relu(a_q - 1))."""
    name = "D1CUBE_ANT"
    for op in _dops.OPS:
        if op.name == name:
            return op

    def _ref(in0, in1, s0, s1, imm2):
        a = in0.astype(np.float32)
        m = np.minimum(a, np.float32(1.0))
        return (m * m * m + (a - m) * a * np.float32(s0)).astype(np.float32)

    m = minn(Src0, One)
    spec = Spec(body=sq(m) * m + (Src0 - m) * Src0 * C0, reference=_ref)
    opcode = _dops._CUSTOM_DVE_ROW_BASE + len(_dops.OPS)
    assert opcode < 0x20
    shas = {}
    for ver in ("v3", "v4"):
        try:
            shas[ver] = _DveOpSpec(
                name=name, opcode=opcode, uops=_dve_lower(spec, ver=ver),
                rd1_en=False).sha(ver)
        except Exception:
            pass
    op = _dops.DveOp(name, spec, subdim=False, uops_sha=shas)
    _dops.OPS.append(op)
    _dops.CUSTOM_DVE_SPECS[name] = spec
    _dops._SUB_OPCODE_FOR_NAME[name] = opcode
    return op


D1CUBE = _register_d1cube()

_programs = {}


def _build_program():
    nc = bacc.Bacc("TRN2", target_bir_lowering=False, debug=False,
                   num_devices=N_CORES)
    xt_d = nc.dram_tensor("xt", [I, BC], F32, kind="ExternalInput").ap()
    h2_d = nc.dram_tensor("h2", [KT * 128, O], F32R, kind="ExternalInput").ap()
    qb_d = nc.dram_tensor("qb", [128, 8], F32, kind="ExternalInput").ap()
    out_d = nc.dram_tensor("out", [BC, O], F32, kind="ExternalOutput").ap()

    with tile.TileContext(nc) as tc:
        with tc.tile_pool(name="g", bufs=1) as gpool, \
             tc.tile_pool(name="x", bufs=4) as xpool, \
             tc.tile_pool(name="a", bufs=4) as apool, \
             tc.tile_pool(name="p", bufs=6) as ppool, \
             tc.tile_pool(name="o", bufs=4) as opool, \
             tc.tile_pool(name="ps", bufs=1, space="PSUM") as pspool:

            # warm tile: memset'd (no DMA dep) -- feeds the scalar table-load
            # hoist and the PE HAM warm-up matmuls before real data lands.
            warm = gpool.tile([128, 512], F32)
            nc.gpsimd.memset(warm[:], 0.0)
            scr = gpool.tile([128, 8], F32)

            qb_s = gpool.tile([128, 8], F32)
            xs = [xpool.tile([128, BC], F32, name=f"xs{ib}", tag=f"xs{ib}")
                  for ib in range(IB)]

            # scalar queue: xs0 + qb first (scalar preamble retires earliest),
            # then the dummy activation forces ACT_TABLE_LOAD before the DMAs
            # complete.
            nc.scalar.dma_start(out=xs[0][:], in_=xt_d[0:128, :])
            nc.scalar.dma_start(out=qb_s[:], in_=qb_d[:])
            nc.scalar.activation(scr[:], warm[:, 0:8], ACTF.Relu, scale=1.0)

            psums = [pspool.tile([128, O], F32, name=f"ps{bb}", tag=f"ps{bb}")
                     for bb in range(NBB)]

            # PE HAM warm-up: ~8 matmuls on the memset tile starting ~7us so
            # the clock gate opens before the first real matmul issues.
            for _ in range(8):
                nc.tensor.matmul(psums[0][:], warm[:, 0:128], warm[:],
                                 start=True, stop=True)

            # sync queue: h2 finely chunked up front, then coarse; xs1-3
            # interleaved where their deadlines fall.
            h2_s = gpool.tile([128, KT, O], F32R)

            def h2_dma(k0, k1):
                nc.sync.dma_start(
                    out=h2_s[:, k0:k1, :],
                    in_=h2_d[k0 * 128:k1 * 128, :].rearrange(
                        "(kt p) o -> p kt o", p=128))

            h2_dma(0, 1)
            h2_dma(1, 2)
            nc.sync.dma_start(out=xs[1][:], in_=xt_d[128:256, :])
            h2_dma(2, 7)
            nc.sync.dma_start(out=xs[2][:], in_=xt_d[256:384, :])
            nc.sync.dma_start(out=xs[3][:], in_=xt_d[384:512, :])
            h2_dma(7, 14)
            h2_dma(14, 21)
            h2_dma(21, 28)

            for ib in range(IB):
                for q in range(Q):
                    kt = ib * Q + q
                    a = apool.tile([128, BC], F32, tag="a")
                    nc.scalar.activation(a[:], xs[ib][:], ACTF.Relu,
                                         bias=qb_s[:, q:q + 1], scale=1.0)
                    p = ppool.tile([128, BC], F32R, tag="p")
                    nc.vector._custom_dve(D1CUBE, out=p[:], in0=a[:], s0=3.0)
                    rhs = h2_s[:, kt, :]
                    for bb in range(NBB):
                        nc.tensor.matmul(psums[bb][:],
                                         p[:, bb * 128:(bb + 1) * 128],
                                         rhs,
                                         start=(kt == 0), stop=(kt == KT - 1))

            # drain: pair psum banks per out tile (half the out-DMA triggers),
            # copies split across scalar and vector.
            for j in range(NBB // 2):
                o2 = opool.tile([128, 2, O], F32, tag="o")
                nc.scalar.copy(o2[:, 0, :], psums[2 * j][:])
                nc.vector.tensor_copy(out=o2[:, 1, :], in_=psums[2 * j + 1][:])
                nc.sync.dma_start(
                    out=out_d[2 * j * 128:(2 * j + 2) * 128, :].rearrange(
                        "(k p) o -> p k o", p=128),
                    in_=o2[:])

    nc.compile()
    return nc


def _get_program():
    if "p" not in _programs:
        _programs["p"] = _build_program()
    return _programs["p"]


def _host_prep(x, weights, coefficients):
    x = np.asarray(x, dtype=np.float32)
    weights = np.asarray(weights, dtype=np.float32)
    coefficients = np.asarray(coefficients, dtype=np.float32)

    # raw truncated-power coefficients G_q = sum_g w5[q-g]/6 * C2_g
    c2 = coefficients.astype(np.float64) * weights.astype(np.float64)[:, :, None]
    c2 = c2.transpose(2, 1, 0)                     # [G, I, O]
    w5 = np.array([1.0, -4.0, 6.0, -4.0, 1.0]) / 6.0
    graw = np.zeros((Q, I, O), dtype=np.float64)
    for q in range(Q):
        for g in range(G):
            r = q - g
            if 0 <= r <= 4:
                graw[q] += w5[r] * c2[g]
    # planes P_q = D1_q = V_q - V_{q+1} (V_7 := 0)  =>  H = A^{-T} G
    A = np.eye(Q)
    A[np.arange(Q - 1), np.arange(1, Q)] = -1.0
    h = np.einsum('pq,qio->pio', np.linalg.inv(A).T, graw)
    # device row order kt = ib*7 + q
    h2k = np.empty((KT, 128, O), dtype=np.float32)
    for ib in range(IB):
        for q in range(Q):
            h2k[ib * Q + q] = h[q, ib * 128:(ib + 1) * 128, :]
    h2k = np.ascontiguousarray(h2k.reshape(KT * 128, O))

    # tp = min(3.5*x, 3.5): t<0 needs no clamp (relu zeroes every plane)
    tp = np.minimum(3.5 * x, np.float32(3.5)).astype(np.float32)
    xt = np.ascontiguousarray(tp.T)                # [I, B]
    qb = np.tile((3.5 - np.arange(8, dtype=np.float32))[None, :], (128, 1))

    in_maps = []
    for c in range(N_CORES):
        in_maps.append({
            "xt": np.ascontiguousarray(xt[:, c * BC:(c + 1) * BC]),
            "h2": h2k,
            "qb": qb,
        })
    return in_maps


def _run(x, weights, coefficients, **spmd_kwargs):
    nc = _get_program()
    in_maps = _host_prep(x, weights, coefficients)
    res = run_bass_kernel_spmd(nc, in_maps, list(range(N_CORES)), **spmd_kwargs)
    out = np.concatenate([res.results[c]["out"] for c in range(N_CORES)], axis=0)
    return out.astype(np.float32), res


def kernel(x, weights, coefficients):
    out, _ = _run(x, weights, coefficients)
    return out


# revision 8
# speedup vs baseline: 2.0983x; 1.0434x over previous
"""Trainium2 Bass kernel for nn_CustomBSplineLayer.

Math: out[b,o] = sum_{i,g} coeff[o,i,g] * w[o,i] * s_g(clip(x[b,i], -1, 1))
where s_g is a cubic B-spline basis on uniform knots (t = 3.5*(x+1) in [0,7],
8 basis functions; s_7 == 0 on the clipped domain).

Uniform-knot truncated-power identity: with V_q = relu(t-q)^3, the layer is
out = sum_{q,i} P_q[b,i] * H[(q,i), o] for ANY plane basis P spanning {V_q}
(H solved exactly on host).  The PE runs float32r (full rate) which rounds
each product at ~2^-12 relative, so per-plane error scales with |P_q|*|H_q|.
First-difference planes D1_q = V_q - V_{q+1} (<=127; V_7 := 0) measure ~5e-3
relative output error -- well under the 2e-2 gate -- and have a key property:

    D1_q is a function of a_q = relu(t-q) ALONE:
        D1_q = m^3 + 3*(a_q - m)*a_q,   m = min(a_q, 1)
    (for t>=q+1 this is 3s^2-3s+1 with s=t-q; for t in [q,q+1] it's s^3).

So each plane needs exactly TWO on-chip ops: one ScalarE relu (free bias
shift) and one fused custom DVE instruction (D1CUBE_ANT, registered below,
5 ALU stages).  No folds, no gpsimd, no squares: per i-block the old kernel's
~30 elementwise ops become 14.  The clamp tp = min(3.5x, 3.5) is precomputed
on host (t<0 needs no clamp: every plane vanishes there via the relu).

Layout (data-parallel over batch, 8 cores x 1024 rows):
  - xt = host tp, pre-transposed: [512 i, 1024 b] per core, i on partitions.
  - planes per (i-block, q): [128, 1024] f32r tiles; matmul lhsT slices are
    [128 K, 128 M=batch] column windows; rhs H tiles [128, 512 o] (f32r).
  - h2 DMA'd in 28 per-kt chunks so the first matmul can start ~1us in.
  - PSUM [128 b, 512 o] x 8 banks accumulate all 28 k-tiles.
"""

import numpy as np

import concourse.mybir as mybir
from concourse import bacc
import concourse.tile as tile
from concourse.bass_utils import run_bass_kernel_spmd
from concourse import dve_ops as _dops
from concourse.dve_spec import Spec, Src0, C0, One, minn, sq
from concourse.dve_spec import lower as _dve_lower
from concourse.dve_uop import DveOpSpec as _DveOpSpec

F32 = mybir.dt.float32
F32R = mybir.dt.float32r
ACTF = mybir.ActivationFunctionType

N_CORES = 8
BATCH, I, O, G = 8192, 512, 512, 8
BC = BATCH // N_CORES          # 1024 batch rows per core
Q = 7                          # planes q = 0..6
IB = I // 128                  # 4 i-blocks
KT = Q * IB                    # 28 k-tiles of 128
NBB = BC // 128                # 8 batch blocks of 128


def _register_d1cube():
    """Register the fused plane op: out = m^3 + s0*(a-m)*a, m = min(a, 1).

    With a = relu(t-q) >= 0 and s0 = 3.0 this is exactly
    D1_q(t) = relu(t-q)^3 - relu(t-q-1)^3 for t <= q+... (all t; V_{q+1}
    is a function of a_q since relu(t-q-1) = relu(a_q - 1))."""
    name = "D1CUBE_ANT"
    for op in _dops.OPS:
        if op.name == name:
            return op

    def _ref(in0, in1, s0, s1, imm2):
        a = in0.astype(np.float32)
        m = np.minimum(a, np.float32(1.0))
        return (m * m * m + (a - m) * a * np.float32(s0)).astype(np.float32)

    m = minn(Src0, One)
    spec = Spec(body=sq(m) * m + (Src0 - m) * Src0 * C0, reference=_ref)
    opcode = _dops._CUSTOM_DVE_ROW_BASE + len(_dops.OPS)
    assert opcode < 0x20
    shas = {}
    for ver in ("v3", "v4"):
        try:
            shas[ver] = _DveOpSpec(
                name=name, opcode=opcode, uops=_dve_lower(spec, ver=ver),
                rd1_en=False).sha(ver)
        except Exception:
            pass
    op = _dops.DveOp(name, spec, subdim=False, uops_sha=shas)
    _dops.OPS.append(op)
    _dops.CUSTOM_DVE_SPECS[name] = spec
    _dops._SUB_OPCODE_FOR_NAME[name] = opcode
    return op


D1CUBE = _register_d1cube()

_programs = {}


def _build_program():
    nc = bacc.Bacc("TRN2", target_bir_lowering=False, debug=False,
                   num_devices=N_CORES)
    xt_d = nc.dram_tensor("xt", [I, BC], F32, kind="ExternalInput").ap()
    h2_d = nc.dram_tensor("h2", [KT * 128, O], F32R, kind="ExternalInput").ap()
    qb_d = nc.dram_tensor("qb", [128, 8], F32, kind="ExternalInput").ap()
    out_d = nc.dram_tensor("out", [BC, O], F32, kind="ExternalOutput").ap()

    with tile.TileContext(nc) as tc:
        with tc.tile_pool(name="g", bufs=1) as gpool, \
             tc.tile_pool(name="x", bufs=4) as xpool, \
             tc.tile_pool(name="a", bufs=4) as apool, \
             tc.tile_pool(name="p", bufs=9) as ppool, \
             tc.tile_pool(name="o", bufs=4) as opool, \
             tc.tile_pool(name="ps", bufs=1, space="PSUM") as pspool:

            # warm tile: memset'd (no DMA dep) -- feeds the scalar table-load
            # hoist and the PE HAM warm-up matmuls before real data lands.
            # f32r so the dummies are single-pass (fp32 lowers to 2x LOW_HIGH).
            warm0 = gpool.tile([128, 512], F32)
            nc.gpsimd.memset(warm0[:], 0.0)
            warm = gpool.tile([128, 512], F32R)
            nc.vector.tensor_copy(out=warm[:], in_=warm0[:])
            scr = gpool.tile([128, 8], F32)

            qb_s = gpool.tile([128, 8], F32)
            xs = [xpool.tile([128, BC], F32, name=f"xs{ib}", tag=f"xs{ib}")
                  for ib in range(IB)]

            # scalar queue: xs0 + qb first (scalar preamble retires earliest),
            # then the dummy activation forces ACT_TABLE_LOAD before the DMAs
            # complete.
            nc.scalar.dma_start(out=xs[0][:], in_=xt_d[0:128, :])
            nc.scalar.dma_start(out=qb_s[:], in_=qb_d[:])
            nc.scalar.activation(scr[:], warm[:, 0:8], ACTF.Relu, scale=1.0)

            psums = [pspool.tile([128, O], F32, name=f"ps{bb}", tag=f"ps{bb}")
                     for bb in range(NBB)]

            # PE HAM warm-up: ~10 single-pass f32r dummies (~4.3us cold) keep
            # the PE busy from ~7.2us until the first real matmul, so the
            # clock gate opens at ~10.6us and the real stream runs at 2.4GHz.
            for _ in range(10):
                nc.tensor.matmul(psums[0][:], warm[:, 0:128], warm[:],
                                 start=True, stop=True)

            # sync queue: h2 finely chunked up front, then coarse; xs1-3
            # interleaved where their deadlines fall.
            h2_s = gpool.tile([128, KT, O], F32R)

            def h2_dma(k0, k1):
                nc.sync.dma_start(
                    out=h2_s[:, k0:k1, :],
                    in_=h2_d[k0 * 128:k1 * 128, :].rearrange(
                        "(kt p) o -> p kt o", p=128))

            h2_dma(0, 1)
            h2_dma(1, 2)
            nc.sync.dma_start(out=xs[1][:], in_=xt_d[128:256, :])
            h2_dma(2, 7)
            nc.sync.dma_start(out=xs[2][:], in_=xt_d[256:384, :])
            nc.sync.dma_start(out=xs[3][:], in_=xt_d[384:512, :])
            h2_dma(7, 14)
            h2_dma(14, 21)
            h2_dma(21, 28)

            # i-blocks 0..2: plane-major (kt inner order), PSUM-bank inner.
            planes = {}
            for ib in range(IB):
                for q in range(Q):
                    kt = ib * Q + q
                    a = apool.tile([128, BC], F32, tag="a")
                    nc.scalar.activation(a[:], xs[ib][:], ACTF.Relu,
                                         bias=qb_s[:, q:q + 1], scale=1.0)
                    p = ppool.tile([128, BC], F32R, tag="p")
                    nc.vector._custom_dve(D1CUBE, out=p[:], in0=a[:], s0=3.0)
                    if ib < IB - 1:
                        rhs = h2_s[:, kt, :]
                        for bb in range(NBB):
                            nc.tensor.matmul(psums[bb][:],
                                             p[:, bb * 128:(bb + 1) * 128],
                                             rhs,
                                             start=(kt == 0), stop=False)
                    else:
                        planes[q] = p

            # last i-block: bank-major so each PSUM bank finishes ~1.6us
            # apart and its drain + out-DMA overlaps the remaining matmuls.
            for bb in range(NBB):
                for q in range(Q):
                    kt = (IB - 1) * Q + q
                    nc.tensor.matmul(psums[bb][:],
                                     planes[q][:, bb * 128:(bb + 1) * 128],
                                     h2_s[:, kt, :],
                                     start=False, stop=(q == Q - 1))
                if bb % 2 == 1:
                    j = bb // 2
                    o2 = opool.tile([128, 2, O], F32, tag="o")
                    nc.scalar.copy(o2[:, 0, :], psums[2 * j][:])
                    nc.vector.tensor_copy(out=o2[:, 1, :],
                                          in_=psums[2 * j + 1][:])
                    nc.sync.dma_start(
                        out=out_d[2 * j * 128:(2 * j + 2) * 128, :].rearrange(
                            "(k p) o -> p k o", p=128),
                        in_=o2[:])

    nc.compile()
    return nc


def _get_program():
    if "p" not in _programs:
        _programs["p"] = _build_program()
    return _programs["p"]


def _host_prep(x, weights, coefficients):
    x = np.asarray(x, dtype=np.float32)
    weights = np.asarray(weights, dtype=np.float32)
    coefficients = np.asarray(coefficients, dtype=np.float32)

    # raw truncated-power coefficients G_q = sum_g w5[q-g]/6 * C2_g
    c2 = coefficients.astype(np.float64) * weights.astype(np.float64)[:, :, None]
    c2 = c2.transpose(2, 1, 0)                     # [G, I, O]
    w5 = np.array([1.0, -4.0, 6.0, -4.0, 1.0]) / 6.0
    graw = np.zeros((Q, I, O), dtype=np.float64)
    for q in range(Q):
        for g in range(G):
            r = q - g
            if 0 <= r <= 4:
                graw[q] += w5[r] * c2[g]
    # planes P_q = D1_q = V_q - V_{q+1} (V_7 := 0)  =>  H = A^{-T} G
    A = np.eye(Q)
    A[np.arange(Q - 1), np.arange(1, Q)] = -1.0
    h = np.einsum('pq,qio->pio', np.linalg.inv(A).T, graw)
    # device row order kt = ib*7 + q
    h2k = np.empty((KT, 128, O), dtype=np.float32)
    for ib in range(IB):
        for q in range(Q):
            h2k[ib * Q + q] = h[q, ib * 128:(ib + 1) * 128, :]
    h2k = np.ascontiguousarray(h2k.reshape(KT * 128, O))

    # tp = min(3.5*x, 3.5): t<0 needs no clamp (relu zeroes every plane)
    tp = np.minimum(3.5 * x, np.float32(3.5)).astype(np.float32)
    xt = np.ascontiguousarray(tp.T)                # [I, B]
    qb = np.tile((3.5 - np.arange(8, dtype=np.float32))[None, :], (128, 1))

    in_maps = []
    for c in range(N_CORES):
        in_maps.append({
            "xt": np.ascontiguousarray(xt[:, c * BC:(c + 1) * BC]),
            "h2": h2k,
            "qb": qb,
        })
    return in_maps


def _run(x, weights, coefficients, **spmd_kwargs):
    nc = _get_program()
    in_maps = _host_prep(x, weights, coefficients)
    res = run_bass_kernel_spmd(nc, in_maps, list(range(N_CORES)), **spmd_kwargs)
    out = np.concatenate([res.results[c]["out"] for c in range(N_CORES)], axis=0)
    return out.astype(np.float32), res


def kernel(x, weights, coefficients):
    out, _ = _run(x, weights, coefficients)
    return out
